# revision 1
# baseline (speedup 1.0000x reference)
"""Causal single-head attention (B=4, S=2048, D=1024) on 8 TRN2 NeuronCores.

Sharding: core c -> (batch b = c//2, half h = c%2). Every core runs the SAME
program: 8 query tiles of 128 rows whose padded causal key-lengths are
L_s = 256*(s+1) for s=0..7.  Core (b, h) takes global query rows
[256*s + 128*h, 256*s + 128*h + 128) of batch b for slot s.  The last 256 key
columns of each score tile get a data-driven causal mask (depends only on h).

All matmuls run in float32r (TF32-like) with fp32 PSUM accumulation.
Phase order K -> V -> Q -> attention keeps K^T, V and Q^T simultaneously
resident without spilling (Q^T is built last, with W_q streamed per output
chunk).
"""

import numpy as np

import concourse.bacc as bacc
import concourse.mybir as mybir
import concourse.tile as tile
from concourse import bass_utils

B, S, D = 4, 2048, 1024
P = 128
DC = D // P          # 8 contraction chunks
EC = D // P          # 8 output-dim chunks
NSLOT = 8            # q tiles per core
NQ = NSLOT * P       # 1024 q rows per core
SCALE = 1.0 / float(np.sqrt(np.float32(S)))
NEG = -1.0e9

F32 = mybir.dt.float32
F32R = mybir.dt.float32r
BF16 = mybir.dt.bfloat16


def build_attention_nc():
    nc = bacc.Bacc("TRN2", target_bir_lowering=False)

    xq = nc.dram_tensor("xq", [NQ, D], F32R, kind="ExternalInput")
    xk = nc.dram_tensor("xk", [S, D], F32R, kind="ExternalInput")
    xv = nc.dram_tensor("xv", [S, D], F32R, kind="ExternalInput")
    wq = nc.dram_tensor("wq", [EC, P, DC, P], F32R, kind="ExternalInput")
    wk = nc.dram_tensor("wk", [EC, P, DC, P], F32R, kind="ExternalInput")
    wv = nc.dram_tensor("wv", [EC, P, DC, P], F32R, kind="ExternalInput")
    mask = nc.dram_tensor("mask", [P, 256], BF16, kind="ExternalInput")
    ident_in = nc.dram_tensor("ident", [P, P], F32R, kind="ExternalInput")
    out = nc.dram_tensor("out", [NQ, D], F32, kind="ExternalOutput")



    with tile.TileContext(nc) as tc:
        with (
            tc.tile_pool(name="res", bufs=1) as res,
            tc.tile_pool(name="xrowq", bufs=3) as xrowqp,
        ):
            kt_sb = res.tile([P, EC, S], F32R)      # K^T  [e, keys]
            v_sb = res.tile([P, S // P, D], F32R)   # V    [keys, e]
            ident = res.tile([P, P], F32R)
            nc.scalar.dma_start(ident, ident_in[:, :])

            # ================= K / V projection phases =================
            with (
                tc.tile_pool(name="wpool", bufs=1) as wpool,
                tc.tile_pool(name="xrow", bufs=2) as xrowp,
                tc.tile_pool(name="xt", bufs=2) as xtp,
                tc.tile_pool(name="pp", bufs=5, space="PSUM") as pp,
                tc.tile_pool(name="pt", bufs=3, space="PSUM") as pt,
            ):

                def load_xT(x_dram, r0, width, split_first=False):
                    """Transpose `width` rows of X starting at r0 into [P, DC, width]."""
                    blk = xtp.tile([P, DC, width], F32R, tag=f"xtblk{width}")
                    for st in range(width // P):
                        xrow = xrowp.tile([P, D], F32R, tag="xrow")
                        rlo = r0 + st * P
                        if split_first:
                            nc.sync.dma_start(
                                xrow[:, 0:512], x_dram[rlo : rlo + P, 0:512]
                            )
                            nc.sync.dma_start(
                                xrow[:, 512:D], x_dram[rlo : rlo + P, 512:D]
                            )
                        else:
                            nc.sync.dma_start(xrow, x_dram[rlo : rlo + P, :])
                        for dc4 in range(2):
                            ptile = pt.tile([P, 4, P], F32R, tag="ptr")
                            for i in range(4):
                                dc = dc4 * 4 + i
                                nc.tensor.transpose(
                                    ptile[:, i, :],
                                    xrow[:, dc * P : (dc + 1) * P],
                                    ident,
                                )
                            nc.vector.tensor_copy(
                                blk[:, dc4 * 4 : dc4 * 4 + 4, st * P : (st + 1) * P],
                                ptile,
                            )
                    return blk

                w_sb = wpool.tile([P, EC, DC, P], F32R, tag="w")

                def load_w(w_t):
                    """W load chunked by output columns, on the ACT HWDGE queue.

                    Reuses the same tile across phases: per-ec WAR deps let the
                    next phase's chunks stream in as the old ones retire.
                    w_sb layout: [p, ec, dc, q] with e = ec*128 + q; each per-ec
                    chunk is a contiguous 4KB run per partition on both sides."""
                    for ec in range(EC):
                        nc.scalar.dma_start(w_sb[:, ec], w_t[ec])
                    return w_sb

                # ---- phase K: K^T resident ----
                blk = load_xT(xk, 0, 512, split_first=True)
                for ec in range(2):
                    nc.scalar.dma_start(w_sb[:, ec], wk[ec])
                for kb in range(S // 512):
                    if kb > 0:
                        blk = load_xT(xk, kb * 512, 512)
                    for ec in range(EC):
                        ps = pp.tile([P, 512], F32, tag="pmm")
                        for half in range(2):
                            for dc in range(DC):
                                nc.tensor.matmul(
                                    ps[:, half * 256 : (half + 1) * 256],
                                    w_sb[:, ec, dc, :],
                                    blk[:, dc, half * 256 : (half + 1) * 256],
                                    start=(dc == 0),
                                    stop=(dc == DC - 1),
                                )
                        nc.scalar.copy(
                            kt_sb[:, ec, kb * 512 : (kb + 1) * 512], ps
                        )
                        if kb == 0 and ec < EC - 2:
                            nc.scalar.dma_start(w_sb[:, ec + 2], wk[ec + 2])

                # ---- phase V: V resident ----
                load_w(wv)
                for kb in range(S // 512):
                    blk = load_xT(xv, kb * 512, 512)
                    for st in range(4):
                        kc = kb * 4 + st
                        for eh in range(2):
                            ps = pp.tile([P, 512], F32, tag="pmm")
                            for dc in range(DC):
                                nc.tensor.matmul(
                                    ps,
                                    blk[:, dc, st * P : (st + 1) * P],
                                    w_sb[:, eh * 4 : (eh + 1) * 4, dc, :],
                                    start=(dc == 0),
                                    stop=(dc == DC - 1),
                                )
                            nc.scalar.copy(
                                v_sb[:, kc, eh * 512 : (eh + 1) * 512], ps
                            )

            # ========== Q projection phase (Q^T resident, W streamed) ==========
            with tc.tile_pool(name="qtp", bufs=1) as qtp:
              qt_sb = qtp.tile([P, EC, NQ], F32R)     # Q^T  [e, q]
              with (
                tc.tile_pool(name="xtq", bufs=1) as xtqp,
                tc.tile_pool(name="wqp", bufs=1) as wqp,
                tc.tile_pool(name="ppq", bufs=4, space="PSUM") as ppq,
                tc.tile_pool(name="ptq", bufs=4, space="PSUM") as ptq,
              ):
                w_sb = wqp.tile([P, EC, DC, P], F32R, tag="wq")
                for bi in range(4):
                    blk = xtqp.tile([P, DC, 256], F32R, tag="xtq")
                    for st in range(2):
                        r0 = bi * 256 + st * P
                        if bi == 0:
                            for ec4 in range(4):
                                ec = st * 4 + ec4
                                nc.scalar.dma_start(w_sb[:, ec], wq[ec])
                        for dc4 in range(2):
                            xrow = xrowqp.tile([P, 512], F32R, tag="xrowq")
                            nc.sync.dma_start(
                                xrow, xq[r0 : r0 + P, dc4 * 512 : (dc4 + 1) * 512]
                            )
                            ptile = ptq.tile([P, 4, P], F32R, tag="ptrq")
                            for i in range(4):
                                nc.tensor.transpose(
                                    ptile[:, i, :],
                                    xrow[:, i * P : (i + 1) * P],
                                    ident,
                                )
                            nc.vector.tensor_copy(
                                blk[:, dc4 * 4 : dc4 * 4 + 4, st * P : (st + 1) * P],
                                ptile,
                            )
                    for ec in range(EC):
                        ps = ppq.tile([P, 256], F32, tag="pmq")
                        for dc in range(DC):
                            nc.tensor.matmul(
                                ps,
                                w_sb[:, ec, dc, :],
                                blk[:, dc, :],
                                start=(dc == 0),
                                stop=(dc == DC - 1),
                            )
                        nc.scalar.copy(
                            qt_sb[:, ec, bi * 256 : (bi + 1) * 256], ps
                        )

              # ================= attention phase =================
              with (
                  tc.tile_pool(name="attn", bufs=2) as attnp,
                  tc.tile_pool(name="psc", bufs=4, space="PSUM") as psc,
                  tc.tile_pool(name="pta", bufs=2, space="PSUM") as pta,
                  tc.tile_pool(name="po", bufs=2, space="PSUM") as po,
              ):
                  mask_sb = attnp.tile([P, 256], BF16, tag="mask")
                  nc.sync.dma_start(mask_sb, mask[:, :])
                  for s in range(NSLOT):
                      L = 256 * (s + 1)
                      nj = (L + 511) // 512
                      nt = L // P

                      attn_sb = attnp.tile([P, S], F32R, tag="attn")
                      acc = attnp.tile([P, 4], F32, tag="acc")
                      ps_list = []
                      for j in range(nj):
                          w_j = min(512, L - j * 512)
                          ps = psc.tile([P, 512], F32, tag="ps_sc")
                          ps_list.append((ps, w_j))
                      for ec in range(EC):
                          for j, (ps, w_j) in enumerate(ps_list):
                              nc.tensor.matmul(
                                  ps[:, :w_j],
                                  qt_sb[:, ec, s * P : (s + 1) * P],
                                  kt_sb[:, ec, j * 512 : j * 512 + w_j],
                                  start=(ec == 0),
                                  stop=(ec == EC - 1),
                              )
                      # causal mask on the last 256 key columns
                      ps_last, w_last = ps_list[-1]
                      off = w_last - 256
                      nc.vector.tensor_add(
                          out=ps_last[:, off : off + 256],
                          in0=ps_last[:, off : off + 256],
                          in1=mask_sb,
                      )
                      # exp + per-chunk row sums
                      for j, (ps, w_j) in enumerate(ps_list):
                          nc.scalar.activation(
                              out=attn_sb[:, j * 512 : j * 512 + w_j],
                              in_=ps[:, :w_j],
                              func=mybir.ActivationFunctionType.Exp,
                              scale=SCALE,
                              accum_out=acc[:, j : j + 1],
                          )
                      total = attnp.tile([P, 1], F32, tag="total")
                      nc.vector.tensor_reduce(
                          total,
                          acc[:, :nj],
                          axis=mybir.AxisListType.X,
                          op=mybir.AluOpType.add,
                      )
                      rec = attnp.tile([P, 1], F32, tag="rec")
                      nc.vector.reciprocal(rec, total)

                      # transpose attn -> attnT [keys, q]
                      attnT = attnp.tile([P, S // P, P], F32R, tag="attnT")
                      for t4 in range((nt + 3) // 4):
                          cnt = min(4, nt - t4 * 4)
                          ptile = pta.tile([P, 4, P], F32R, tag="pta")
                          for i in range(cnt):
                              t = t4 * 4 + i
                              nc.tensor.transpose(
                                  ptile[:, i, :], attn_sb[:, t * P : (t + 1) * P], ident
                              )
                          nc.vector.tensor_copy(
                              attnT[:, t4 * 4 : t4 * 4 + cnt, :], ptile[:, :cnt, :]
                          )

                      # attn @ V, normalized on copy-out
                      out_sb = attnp.tile([P, D], F32, tag="out", bufs=1)
                      for eh in range(2):
                          ps_o = po.tile([P, 512], F32, tag="ps_o")
                          for t in range(nt):
                              nc.tensor.matmul(
                                  ps_o,
                                  attnT[:, t, :],
                                  v_sb[:, t, eh * 512 : (eh + 1) * 512],
                                  start=(t == 0),
                                  stop=(t == nt - 1),
                              )
                          nc.scalar.activation(
                              out=out_sb[:, eh * 512 : (eh + 1) * 512],
                              in_=ps_o,
                              func=mybir.ActivationFunctionType.Copy,
                              scale=rec,
                          )
                      nc.sync.dma_start(out[s * P : (s + 1) * P, :], out_sb)

    nc.compile()
    return nc


_NC_CACHE = None


def _get_nc():
    global _NC_CACHE
    if _NC_CACHE is None:
        _NC_CACHE = build_attention_nc()
    return _NC_CACHE


def _make_mask(h: int) -> np.ndarray:
    import ml_dtypes

    i = np.arange(P)[:, None]
    j = np.arange(256)[None, :]
    allowed = j <= (i + 128 * h)
    return np.where(allowed, 0.0, NEG).astype(ml_dtypes.bfloat16)


def kernel(
    inputs_for_keys,
    inputs_for_values,
    inputs_for_queries,
    weight_K,
    weight_V,
    weight_Q,
    trace=False,
):
    xk_full = np.ascontiguousarray(np.asarray(inputs_for_keys, dtype=np.float32))
    xv_full = np.ascontiguousarray(np.asarray(inputs_for_values, dtype=np.float32))
    xq_full = np.ascontiguousarray(np.asarray(inputs_for_queries, dtype=np.float32))
    def _reorder_w(w):
        w = np.asarray(w, dtype=np.float32).reshape(DC, P, EC, P)
        return np.ascontiguousarray(w.transpose(2, 1, 0, 3))

    w_k = _reorder_w(weight_K)
    w_v = _reorder_w(weight_V)
    w_q = _reorder_w(weight_Q)

    masks = [_make_mask(0), _make_mask(1)]
    ident_np = np.eye(P, dtype=np.float32)
    in_maps = []
    for c in range(2 * B):
        b, h = c // 2, c % 2
        rows = np.concatenate(
            [
                xq_full[b, 256 * s + 128 * h : 256 * s + 128 * h + P, :]
                for s in range(NSLOT)
            ],
            axis=0,
        )
        in_maps.append(
            {
                "xq": np.ascontiguousarray(rows),
                "xk": xk_full[b],
                "xv": xv_full[b],
                "wq": w_q,
                "wk": w_k,
                "wv": w_v,
                "mask": masks[h],
                "ident": ident_np,
            }
        )

    nc = _get_nc()
    res = bass_utils.run_bass_kernel_spmd(
        nc, in_maps, core_ids=list(range(2 * B)), trace=trace
    )

    out = np.empty((B, S, D), dtype=np.float32)
    for c in range(2 * B):
        b, h = c // 2, c % 2
        o = res.results[c]["out"]
        for s in range(NSLOT):
            out[b, 256 * s + 128 * h : 256 * s + 128 * h + P, :] = o[
                s * P : (s + 1) * P, :
            ]

    if trace:
        return out, res
    return out



# revision 6
# speedup vs baseline: 1.2116x; 1.2116x over previous
"""Causal single-head attention (B=4, S=2048, D=1024) on 8 TRN2 NeuronCores.

Sharding: core c -> (batch b = c//2, half h = c%2). Every core runs the SAME
program: 8 query tiles of 128 rows whose padded causal key-lengths are
L_s = 256*(s+1) for s=0..7.  Core (b, h) takes global query rows
[256*s + 128*h, 256*s + 128*h + 128) of batch b for slot s.  The last 256 key
columns of each score tile get a data-driven causal mask (depends only on h).

All device operands are bf16 (inputs are quantized host-side; PSUM accumulates
fp32).  X / W / Q are shipped pre-transposed / pre-reshaped from the host so
the device never transposes activations: X^T arrives as [DC, P, S] so the
contraction dim d sits on partitions for every projection matmul.

Phases: K^T -> V -> Q^T -> attention, with weights prefetched one phase ahead
and X streamed in 512-column blocks.  Attention slots run longest-first with
per-chunk exp so the softmax pipeline hides under the next chunk's matmuls.
"""

import numpy as np

import concourse.bacc as bacc
import concourse.mybir as mybir
import concourse.tile as tile
from concourse import bass_utils

B, S, D = 4, 2048, 1024
P = 128
DC = D // P          # 8 contraction chunks
EC = D // P          # 8 output-dim chunks
NSLOT = 8            # q tiles per core
NQ = NSLOT * P       # 1024 q rows per core
SCALE = 1.0 / float(np.sqrt(np.float32(S)))
NEG = -1.0e9

F32 = mybir.dt.float32
BF16 = mybir.dt.bfloat16


def build_attention_nc():
    nc = bacc.Bacc("TRN2", target_bir_lowering=False)

    xqT = nc.dram_tensor("xqT", [P, DC, NQ], BF16, kind="ExternalInput")
    xkT = nc.dram_tensor("xkT", [P, DC, S], BF16, kind="ExternalInput")
    xvT = nc.dram_tensor("xvT", [P, DC, S], BF16, kind="ExternalInput")
    wq = nc.dram_tensor("wq", [DC, P, D], BF16, kind="ExternalInput")
    wk = nc.dram_tensor("wk", [DC, P, D], BF16, kind="ExternalInput")
    wv = nc.dram_tensor("wv", [DC, P, D], BF16, kind="ExternalInput")
    mask = nc.dram_tensor("mask", [P, 256], BF16, kind="ExternalInput")
    ident_in = nc.dram_tensor("ident", [P, P], BF16, kind="ExternalInput")
    out = nc.dram_tensor("out", [NQ, D], F32, kind="ExternalOutput")

    with tile.TileContext(nc) as tc:
        with tc.tile_pool(name="res", bufs=1) as res:
            kt_sb = res.tile([P, EC, S], BF16)      # K^T  [e, keys]
            v_sb = res.tile([P, S // P, D], BF16)   # V    [keys, e]
            qt_sb = res.tile([P, EC, NQ], BF16)     # Q^T  [e, q]
            ident = res.tile([P, P], BF16)
            mask_sb = res.tile([P, 256], BF16)
            nc.sync.dma_start(ident, ident_in[:, :])
            nc.sync.dma_start(mask_sb, mask[:, :])

            # ============ projection phases (K^T, V, Q^T) ============
            with (
                tc.tile_pool(name="wp", bufs=2) as wp,
                tc.tile_pool(name="xp", bufs=3) as xp,
                tc.tile_pool(name="pp", bufs=4, space="PSUM") as pp,
            ):
                def load_w(w_t):
                    w_sb = wp.tile([P, DC, D], BF16, tag="w")
                    for dc in range(DC):
                        nc.gpsimd.dma_start(w_sb[:, dc], w_t[dc])
                    return w_sb

                def load_x(x_t, c0, width):
                    xb = xp.tile([P, DC, width], BF16, tag="x")
                    nc.sync.dma_start(xb[:, 0:4], x_t[:, 0:4, c0 : c0 + width])
                    nc.sync.dma_start(xb[:, 4:8], x_t[:, 4:8, c0 : c0 + width])
                    return xb

                wk_sb = load_w(wk)
                wv_sb = load_w(wv)

                # ---- K phase: K^T resident ----
                for kb in range(S // 512):
                    xb = load_x(xkT, kb * 512, 512)
                    for ec in range(EC):
                        ps = pp.tile([P, 512], F32, tag="ps")
                        for dc in range(DC):
                            nc.tensor.matmul(
                                ps,
                                wk_sb[:, dc, ec * P : (ec + 1) * P],
                                xb[:, dc],
                                start=(dc == 0),
                                stop=(dc == DC - 1),
                            )
                        if ec % 2 == 0:
                            nc.vector.tensor_copy(
                                kt_sb[:, ec, kb * 512 : (kb + 1) * 512], ps
                            )
                        else:
                            nc.scalar.copy(
                                kt_sb[:, ec, kb * 512 : (kb + 1) * 512], ps
                            )

                # ---- V phase: V resident ----
                wq_sb = load_w(wq)
                for kb in range(S // 512):
                    xb = load_x(xvT, kb * 512, 512)
                    for st in range(4):
                        kc = kb * 4 + st
                        for eh in range(2):
                            ps = pp.tile([P, 512], F32, tag="ps")
                            for dc in range(DC):
                                nc.tensor.matmul(
                                    ps,
                                    xb[:, dc, st * P : (st + 1) * P],
                                    wv_sb[:, dc, eh * 512 : (eh + 1) * 512],
                                    start=(dc == 0),
                                    stop=(dc == DC - 1),
                                )
                            if eh == 0:
                                nc.vector.tensor_copy(
                                    v_sb[:, kc, eh * 512 : (eh + 1) * 512], ps
                                )
                            else:
                                nc.scalar.copy(
                                    v_sb[:, kc, eh * 512 : (eh + 1) * 512], ps
                                )

                # ---- Q phase: Q^T resident (qb=1 first; attention starts at
                # slot 7, whose queries live in the upper half) ----
                for qb in (1, 0):
                    xb = load_x(xqT, qb * 512, 512)
                    for ec in range(EC):
                        ps = pp.tile([P, 512], F32, tag="ps")
                        for dc in range(DC):
                            nc.tensor.matmul(
                                ps,
                                wq_sb[:, dc, ec * P : (ec + 1) * P],
                                xb[:, dc],
                                start=(dc == 0),
                                stop=(dc == DC - 1),
                            )
                        if ec % 2 == 0:
                            nc.vector.tensor_copy(
                                qt_sb[:, ec, qb * 512 : (qb + 1) * 512], ps
                            )
                        else:
                            nc.scalar.copy(
                                qt_sb[:, ec, qb * 512 : (qb + 1) * 512], ps
                            )

            # ================= attention phase =================
            with (
                tc.tile_pool(name="attn", bufs=2) as attnp,
                tc.tile_pool(name="psc", bufs=3, space="PSUM") as psc,
                tc.tile_pool(name="pta", bufs=2, space="PSUM") as pta,
                tc.tile_pool(name="po", bufs=2, space="PSUM") as po,
            ):
                for s in range(NSLOT - 1, -1, -1):
                    L = 256 * (s + 1)
                    nj = (L + 511) // 512
                    nt = L // P

                    attn_sb = attnp.tile([P, S], BF16, tag="attn")
                    acc = attnp.tile([P, 4], F32, tag="acc")
                    for j in range(nj):
                        w_j = min(512, L - j * 512)
                        ps = psc.tile([P, 512], F32, tag="ps_sc")
                        for ec in range(EC):
                            nc.tensor.matmul(
                                ps[:, :w_j],
                                qt_sb[:, ec, s * P : (s + 1) * P],
                                kt_sb[:, ec, j * 512 : j * 512 + w_j],
                                start=(ec == 0),
                                stop=(ec == EC - 1),
                            )
                        if j == nj - 1:
                            # causal mask on the last 256 key columns
                            off = w_j - 256
                            nc.vector.tensor_add(
                                out=ps[:, off : off + 256],
                                in0=ps[:, off : off + 256],
                                in1=mask_sb,
                            )
                        nc.scalar.activation(
                            out=attn_sb[:, j * 512 : j * 512 + w_j],
                            in_=ps[:, :w_j],
                            func=mybir.ActivationFunctionType.Exp,
                            scale=SCALE,
                            accum_out=acc[:, j : j + 1],
                        )
                    total = attnp.tile([P, 1], F32, tag="total")
                    nc.vector.tensor_reduce(
                        total,
                        acc[:, :nj],
                        axis=mybir.AxisListType.X,
                        op=mybir.AluOpType.add,
                    )
                    rec = attnp.tile([P, 1], F32, tag="rec")
                    nc.vector.reciprocal(rec, total)

                    # transpose attn -> attnT [keys, q]
                    attnT = attnp.tile([P, S // P, P], BF16, tag="attnT")
                    for t4 in range((nt + 3) // 4):
                        cnt = min(4, nt - t4 * 4)
                        pt = pta.tile([P, 4, P], BF16, tag="pt")
                        for i in range(cnt):
                            t = t4 * 4 + i
                            nc.tensor.transpose(
                                pt[:, i, :], attn_sb[:, t * P : (t + 1) * P], ident
                            )
                        nc.vector.tensor_copy(
                            attnT[:, t4 * 4 : t4 * 4 + cnt, :], pt[:, :cnt, :]
                        )

                    # attn @ V, normalized on copy-out
                    out_sb = attnp.tile([P, D], F32, tag="out")
                    for eh in range(2):
                        ps_o = po.tile([P, 512], F32, tag="ps_o")
                        for t in range(nt):
                            nc.tensor.matmul(
                                ps_o,
                                attnT[:, t, :],
                                v_sb[:, t, eh * 512 : (eh + 1) * 512],
                                start=(t == 0),
                                stop=(t == nt - 1),
                            )
                        nc.scalar.activation(
                            out=out_sb[:, eh * 512 : (eh + 1) * 512],
                            in_=ps_o,
                            func=mybir.ActivationFunctionType.Copy,
                            scale=rec,
                        )
                    nc.sync.dma_start(out[s * P : (s + 1) * P, :], out_sb)

    nc.compile()
    return nc


_NC_CACHE = None


def _get_nc():
    global _NC_CACHE
    if _NC_CACHE is None:
        _NC_CACHE = build_attention_nc()
    return _NC_CACHE


def _make_mask(h: int) -> np.ndarray:
    import ml_dtypes

    i = np.arange(P)[:, None]
    j = np.arange(256)[None, :]
    allowed = j <= (i + 128 * h)
    return np.where(allowed, 0.0, NEG).astype(ml_dtypes.bfloat16)


def kernel(
    inputs_for_keys,
    inputs_for_values,
    inputs_for_queries,
    weight_K,
    weight_V,
    weight_Q,
    trace=False,
):
    import ml_dtypes

    bf16 = ml_dtypes.bfloat16

    def _xT(x):  # [rows, D] f32 -> [P, DC, rows] bf16 (transposed, p-major)
        xt = np.asarray(x, dtype=np.float32).T.reshape(DC, P, x.shape[0])
        return np.ascontiguousarray(xt.transpose(1, 0, 2)).astype(bf16)

    def _w(w):  # [D, D] f32 -> [DC, P, D] bf16 (d_in on partitions)
        return np.asarray(w, dtype=np.float32).reshape(DC, P, D).astype(bf16)

    xk_full = np.asarray(inputs_for_keys, dtype=np.float32)
    xv_full = np.asarray(inputs_for_values, dtype=np.float32)
    xq_full = np.asarray(inputs_for_queries, dtype=np.float32)

    w_k = _w(weight_K)
    w_v = _w(weight_V)
    w_q = _w(weight_Q)

    xkT = [_xT(xk_full[b]) for b in range(B)]
    xvT = [_xT(xv_full[b]) for b in range(B)]

    masks = [_make_mask(0), _make_mask(1)]
    ident_np = np.eye(P, dtype=np.float32).astype(bf16)
    in_maps = []
    for c in range(2 * B):
        b, h = c // 2, c % 2
        rows = np.concatenate(
            [
                xq_full[b, 256 * s + 128 * h : 256 * s + 128 * h + P, :]
                for s in range(NSLOT)
            ],
            axis=0,
        )
        in_maps.append(
            {
                "xqT": _xT(rows),
                "xkT": xkT[b],
                "xvT": xvT[b],
                "wq": w_q,
                "wk": w_k,
                "wv": w_v,
                "mask": masks[h],
                "ident": ident_np,
            }
        )

    nc = _get_nc()
    res = bass_utils.run_bass_kernel_spmd(
        nc, in_maps, core_ids=list(range(2 * B)), trace=trace
    )

    out = np.empty((B, S, D), dtype=np.float32)
    for c in range(2 * B):
        b, h = c // 2, c % 2
        o = res.results[c]["out"]
        for s in range(NSLOT):
            out[b, 256 * s + 128 * h : 256 * s + 128 * h + P, :] = o[
                s * P : (s + 1) * P, :
            ]

    if trace:
        return out, res
    return out
